# revision 18
# baseline (speedup 1.0000x reference)
"""Distributed MultiHeadAttention kernel for 8 TRN2 NeuronCores.

Problem: B=4, S=2048, D=1024, H=16, DH=64, fp32 reference, full
(non-causal) attention. ~137 GFLOP total.

Sharding (v2): core c owns batch b=c//2 and HEAD-HALF hh=c%2 (8 heads,
full 2048-query x 2048-key attention). Unlike the query-split baseline,
K/V projections are NOT duplicated (each core projects only its own 8
heads' K/V/Q over the full sequence). The output projection produces a
PARTIAL Y[2048,1024] (contraction over the core's 512 ao-dims); the host
sums the two head-half partials per batch and adds bo. One SPMD program;
per-core behavior is entirely encoded in the input tensors (weight
slices per head-half, x^T per batch).

Per-core pipeline (all matmuls fp16/bf16, fp32 PSUM):
- Scores are computed transposed sT[keys, q] per head-pair: h0 contracts
  on partitions 0-63, h1 on 64-127; the two MMs run CONCURRENTLY (PE
  row-group tiling, auto-derived from lhsT base_partition) into one
  [128, 1024] PSUM tile, so ONE scalar Exp per key chunk does the
  softmax numerator (no max subtraction; |s| <~ 35, bf16 P).
- PV uses PE COLUMN-group tiling: h0's V (M=64) lands in array cols
  0-63 and h1's V in cols 64-127 via explicit tile_position=(0,0)/
  (0,64); the two MMs run concurrently, writing aoT for both heads into
  one [128, 512] PSUM tile (h0 rows 0-63, h1 rows 64-127).
- Softmax denominators come from a second col-tiled concurrent MM pair
  with an all-ones [128, 64] stationary: out rows = 64 broadcast copies
  of sum_k P[k, q] per head, accumulated in PSUM across key chunks.
  1/den runs on the VECTOR engine (reciprocal_approx_fast, 18-bit) --
  the scalar engine stays dedicated to Exp (it is the ~276us wall:
  256 x [128,1024] Exp at ~1.077us cadence).
- The attention inner loop needs only ~640ns/kc of tensor time vs
  1077ns/kc of scalar time; projections for later pairs + the V
  projection + per-q2-block output projections are emitted interleaved
  into that slack (pair-0's V projection is split out at N=128 so the
  first exp can issue ~10us into the kernel).
- Output projection runs per 512-query block as soon as pair 3's
  normalize for that block completes, so the tail after the last exp is
  only ~one block of O-proj + DMA.
- walrus in this environment rejects >1 semaphore wait per instruction;
  a post-pass hoists extra waits onto standalone same-engine
  InstEventSemaphore instructions.
"""
import numpy as np
import ml_dtypes
import concourse.bass as bass
import concourse.mybir as mybir
from concourse.tile import TileContext
from concourse.bass_utils import run_bass_kernel_spmd


def _ensure_trace_shim():
    """concourse's axon trace path imports antenv.axon_hooks, which this
    container's antenv lacks. Install a working ctypes-based NTFF hook (or a
    None hook) so BASS_TRACE=1 degrades gracefully instead of crashing."""
    try:
        import antenv.axon_hooks  # noqa: F401
        return
    except ImportError:
        pass
    import sys as _sys
    import types as _types
    hook = None
    try:
        if "/root/.axon_site" not in _sys.path:
            _sys.path.insert(0, "/root/.axon_site")
        from trn_agent_boot.trn_boot import _ntff_profile_via_ctypes
        hook = _ntff_profile_via_ctypes("/opt/axon/libaxon_pjrt.so")
    except Exception:
        hook = None
    mod = _types.ModuleType("antenv.axon_hooks")
    mod.get_axon_ntff_profile_hook = lambda: hook
    mod.set_axon_ntff_profile_hook = lambda h: None
    _sys.modules["antenv.axon_hooks"] = mod
    try:
        import concourse.bass_utils as _bu
        _bu.upload_artifacts = lambda tmpdir: f"local:{tmpdir}"
    except Exception:
        pass


_ensure_trace_shim()


F32 = mybir.dt.float32
F32R = mybir.dt.float32r
BF16 = mybir.dt.bfloat16
FP16 = mybir.dt.float16

B, S, D, H = 4, 2048, 1024, 16
DH = D // H
N_CORES = 8
PAIRS = 4                  # local head pairs per core (8 heads)
DINC = 8                   # 128-wide din chunks
KC = S // 128              # 16 key chunks
QT = S // 512              # 4 query tiles of 512

_ws_counter = 0


def _split_multi_waits(nc):
    """walrus in this env rejects >1 sem wait per instruction; hoist extras
    onto same-engine standalone semaphore-wait instructions."""
    global _ws_counter
    f = nc.m.functions[0]
    for bb in f.blocks:
        insts = bb.instructions  # live list
        i = 0
        while i < len(insts):
            inst = insts[i]
            si = inst.sync_info
            waits = list(si.on_wait) if si is not None and si.on_wait else []
            if len(waits) > 1:
                eng = getattr(inst, "engine", None)
                assert eng is not None and eng in nc.engines, (
                    f"multi-wait on non-engine inst {inst.name} ({type(inst).__name__})"
                )
                for w in waits[:-1]:
                    _ws_counter += 1
                    ev = mybir.InstEventSemaphore(
                        name=f"I-wsplit-{_ws_counter}", ins=[], outs=[]
                    )
                    ev.engine = eng
                    ev.sync_info = mybir.SyncInfo(on_wait=[w], on_update=[])
                    nc.register_instruction(ev, overwrite=True)
                    insts.insert(i, ev)
                    i += 1
                inst.sync_info = mybir.SyncInfo(
                    on_wait=[waits[-1]], on_update=list(si.on_update or [])
                )
            i += 1


def build_bass():
    nc = bass.Bass()
    XT = nc.declare_dram_parameter("XT", [D, S], FP16, isOutput=False)
    WQP = nc.declare_dram_parameter("WQP", [PAIRS, 128, 1024], FP16, isOutput=False)
    WKP = nc.declare_dram_parameter("WKP", [PAIRS, 128, 1024], FP16, isOutput=False)
    WVP0 = nc.declare_dram_parameter("WVP0", [128, 1024], FP16, isOutput=False)
    WVP123 = nc.declare_dram_parameter("WVP123", [128, 3072], FP16, isOutput=False)
    WOP = nc.declare_dram_parameter("WOP", [2, 128, 2048], FP16, isOutput=False)
    BQK = nc.declare_dram_parameter("BQK", [128, 2 * PAIRS], F32, isOutput=False)
    BVB = nc.declare_dram_parameter("BVB", [128, 512], F32, isOutput=False)
    Y = nc.declare_dram_parameter("Y", [S, D], F32, isOutput=True)

    with TileContext(nc) as tc:
        with (
            tc.tile_pool(name="sb", bufs=1) as sb,
            tc.tile_pool(name="ps", bufs=1, space="PSUM") as ps,
        ):
            # ---- constants
            ones64 = sb.tile([128, 64], BF16, tag="ones64")
            nc.vector.memset(ones64[:, :], 1.0)
            bqk = sb.tile([128, 2 * PAIRS], F32, tag="bqk")
            bvb = sb.tile([128, 512], F32, tag="bvb")
            nc.sync.dma_start(out=bqk[:, :], in_=BQK[:, :])
            nc.sync.dma_start(out=bvb[:, :], in_=BVB[:, :])

            # ---- xT in waves: cols 0-511 first (split across two queues),
            # then 512-1023 and 1024-2047 on gpsimd only, keeping the sync
            # queue free for weight loads.
            xt = []
            for d in range(DINC):
                t = sb.tile([128, S], FP16, tag=f"xt{d}")
                eng = nc.sync if d % 2 == 0 else nc.gpsimd
                eng.dma_start(out=t[:, 0:512], in_=XT[d * 128:(d + 1) * 128, 0:512])
                xt.append(t)
            for d in range(DINC):
                nc.gpsimd.dma_start(out=xt[d][:, 512:1024],
                                    in_=XT[d * 128:(d + 1) * 128, 512:1024])
            for d in range(DINC):
                nc.gpsimd.dma_start(out=xt[d][:, 1024:2048],
                                    in_=XT[d * 128:(d + 1) * 128, 1024:2048])

            # ---- weights (sync queue, in priority order)
            wv0_t = sb.tile([128, 1024], FP16, tag="wv0", name="wv0_t")
            nc.sync.dma_start(out=wv0_t[:, :], in_=WVP0[:, :])
            wq_t = [None] * PAIRS
            wk_t = [None] * PAIRS
            wq_t[0] = sb.tile([128, 1024], FP16, tag="wq", bufs=2, name="wq0")
            wk_t[0] = sb.tile([128, 1024], FP16, tag="wk", bufs=2, name="wk0")
            nc.sync.dma_start(out=wq_t[0][:, :], in_=WQP[0, :, :])
            nc.sync.dma_start(out=wk_t[0][:, :], in_=WKP[0, :, :])
            wv123_t = sb.tile([128, 3072], FP16, tag="wv123", name="wv123_t")
            nc.sync.dma_start(out=wv123_t[:, :], in_=WVP123[:, :])

            # V tiles per key chunk: pair0 separate from pairs 1-3 so pair-0
            # attention only depends on the cheap N=128 projection.
            vt0 = [sb.tile([128, 128], BF16, tag=f"vt0_{kc}", name=f"vt0_{kc}") for kc in range(KC)]
            vt123 = [sb.tile([128, 384], BF16, tag=f"vt123_{kc}", name=f"vt123_{kc}") for kc in range(KC)]
            # qt/kt per pair as 4 tiles of [128, 512] (tok granularity)
            qt = [[None] * QT for _ in range(PAIRS)]
            kt = [[None] * QT for _ in range(PAIRS)]
            # aoT per (pair, q2): [128, 512] fp16
            aot = [[sb.tile([128, 512], FP16, tag=f"ao{j}_{q2}", name=f"ao{j}_{q2}")
                    for q2 in range(QT)] for j in range(PAIRS)]

            def emit_vp0(kc):
                # [128, 512] request keeps every "pp" allocation equal-sized
                vps = ps.tile([128, 512], F32, tag="pp")
                for d in range(DINC):
                    nc.tensor.matmul(
                        vps[:, 0:128], xt[d][:, kc * 128:(kc + 1) * 128],
                        wv0_t[:, d * 128:(d + 1) * 128],
                        start=(d == 0), stop=(d == DINC - 1),
                    )
                with nc.allow_low_precision(reason="bf16 V"):
                    nc.vector.tensor_add(vt0[kc][:, :], vps[:, 0:128], bvb[:, 0:128])

            def emit_vp123(kc):
                vps = ps.tile([128, 512], F32, tag="pp")
                for d in range(DINC):
                    nc.tensor.matmul(
                        vps[:, 0:384], xt[d][:, kc * 128:(kc + 1) * 128],
                        wv123_t[:, d * 384:(d + 1) * 384],
                        start=(d == 0), stop=(d == DINC - 1),
                    )
                with nc.allow_low_precision(reason="bf16 V"):
                    nc.vector.tensor_add(vt123[kc][:, :], vps[:, 0:384], bvb[:, 128:512])

            def emit_qproj(j, t):
                if wq_t[j] is None:
                    need(f"wdma{j}")
                # qT tile [128 pair-dims, 512 toks]
                qt[j][t] = sb.tile([128, 512], FP16, tag="qt", bufs=2 * QT, name=f"qt{j}_{t}")
                qps = ps.tile([128, 512], F32, tag="pp")
                for d in range(DINC):
                    nc.tensor.matmul(
                        qps[:, :], wq_t[j][:, d * 128:(d + 1) * 128],
                        xt[d][:, t * 512:(t + 1) * 512],
                        start=(d == 0), stop=(d == DINC - 1),
                    )
                with nc.allow_low_precision(reason="f32r rounding"):
                    nc.vector.tensor_scalar_add(
                        qt[j][t][:, :], qps[:, :], bqk[:, 2 * j:2 * j + 1])

            def emit_kproj(j, t):
                if wk_t[j] is None:
                    need(f"wdma{j}")
                kt[j][t] = sb.tile([128, 512], FP16, tag="kt", bufs=2 * QT, name=f"kt{j}_{t}")
                kps = ps.tile([128, 512], F32, tag="pp")
                for d in range(DINC):
                    nc.tensor.matmul(
                        kps[:, :], wk_t[j][:, d * 128:(d + 1) * 128],
                        xt[d][:, t * 512:(t + 1) * 512],
                        start=(d == 0), stop=(d == DINC - 1),
                    )
                with nc.allow_low_precision(reason="f32r rounding"):
                    nc.vector.tensor_scalar_add(
                        kt[j][t][:, :], kps[:, :], bqk[:, 2 * j + 1:2 * j + 2])

            def emit_wdma(j):
                wq_t[j] = sb.tile([128, 1024], FP16, tag="wq", bufs=2, name=f"wq{j}")
                wk_t[j] = sb.tile([128, 1024], FP16, tag="wk", bufs=2, name=f"wk{j}")
                nc.sync.dma_start(out=wq_t[j][:, :], in_=WQP[j, :, :])
                nc.sync.dma_start(out=wk_t[j][:, :], in_=WKP[j, :, :])

            wo_t = [None, None]

            def emit_wodma():
                for nt in range(2):
                    wo_t[nt] = sb.tile([128, 2048], FP16, tag=f"wo{nt}", name=f"wo{nt}")
                    nc.sync.dma_start(out=wo_t[nt][:, :], in_=WOP[nt, :, :])

            def emit_oproj(q2, tc_, nt, tag="pp"):
                # Y rows [q2*512 + tc_*128 .. +128), cols [nt*512 .. +512)
                yps = ps.tile([128, 512], F32, tag=tag,
                              bufs=2 if tag == "pv" else 1)
                for j in range(PAIRS):
                    nc.tensor.matmul(
                        yps[:, :], aot[j][q2][:, tc_ * 128:(tc_ + 1) * 128],
                        wo_t[nt][:, j * 512:(j + 1) * 512],
                        start=(j == 0), stop=(j == PAIRS - 1),
                    )
                y_sb = sb.tile([128, 512], F32, tag="y", bufs=2)
                nc.vector.tensor_copy(y_sb[:, :], yps[:, :])
                r0 = q2 * 512 + tc_ * 128
                nc.sync.dma_start(out=Y[r0:r0 + 128, nt * 512:(nt + 1) * 512],
                                  in_=y_sb[:, :])

            # ---------- background work queue ----------
            # (key, emit_fn) ordered by when each result is first needed
            # (vt0[kc] at kc, kt0 tile t at kc=4t, qt0 tile t at q2=t,
            # vt123 before pair 1). pump() pushes ~1 item per kc iteration;
            # need() pulls a specific item early (build-time safety: a tile's
            # writers must be emitted before its readers).
            bg = []
            bg.append(("wdma1", lambda: emit_wdma(1)))
            for kc in range(4, KC):
                if kc % 4 == 0:
                    t = kc // 4
                    bg.append((f"k0_{t}", lambda t=t: emit_kproj(0, t)))
                bg.append((f"vp0_{kc}", lambda kc=kc: emit_vp0(kc)))
            for t in range(1, QT):
                bg.append((f"q0_{t}", lambda t=t: emit_qproj(0, t)))
            for kc in range(KC):
                bg.append((f"vp123_{kc}", lambda kc=kc: emit_vp123(kc)))
            bg.append(("wodma", lambda: emit_wodma()))
            # remaining projections are appended per-pair below

            bg_done = set()

            def bg_run(idx):
                key, fn = bg[idx]
                if key not in bg_done:
                    bg_done.add(key)
                    fn()

            bg_i = [0]

            def pump(n=1):
                done = 0
                while bg_i[0] < len(bg) and done < n:
                    if bg[bg_i[0]][0] not in bg_done:
                        bg_run(bg_i[0])
                        done += 1
                    bg_i[0] += 1

            def need(key):
                for idx in range(len(bg)):
                    if bg[idx][0] == key:
                        bg_run(idx)
                        return
                raise KeyError(key)

            # ---------- startup: minimal work before the first exp ----------
            for kc in range(4):
                emit_vp0(kc)
            emit_kproj(0, 0)
            emit_qproj(0, 0)

            # ---------- main attention loops ----------
            for j in range(PAIRS):
                if j + 1 < PAIRS:
                    # queue next pair's projections behind current bg work
                    bg.append((f"k{j+1}_0", lambda j=j: emit_kproj(j + 1, 0)))
                    bg.append((f"q{j+1}_0", lambda j=j: emit_qproj(j + 1, 0)))
                    for t in range(1, QT):
                        bg.append((f"k{j+1}_{t}", lambda j=j, t=t: emit_kproj(j + 1, t)))
                    for t in range(1, QT):
                        bg.append((f"q{j+1}_{t}", lambda j=j, t=t: emit_qproj(j + 1, t)))
                    if j + 2 < PAIRS:
                        bg.append((f"wdma{j+2}", lambda j=j: emit_wdma(j + 2)))
                for q2 in range(QT):
                    # build-time safety pulls (normally already pumped);
                    # per-kc tiles are pulled just-in-time inside the loop
                    if qt[j][q2] is None:
                        need(f"q{j}_{q2}")
                    if j == PAIRS - 1:
                        need("wodma")
                    psC = ps.tile([128, 512], F32, tag="pv", bufs=2)
                    psD = ps.tile([128, 512], F32, tag="pd")
                    qsl = slice(q2 * 512, (q2 + 1) * 512)
                    for kc in range(KC):
                        if kt[j][kc // 4] is None:
                            need(f"k{j}_{kc // 4}")
                        if j == 0 and kc >= 4:
                            need(f"vp0_{kc}")
                        elif j == 1:
                            need(f"vp123_{kc}")
                        pss = ps.tile([128, 1024], F32, tag="ps_s", bufs=2)
                        ksl = slice((kc % 4) * 128, (kc % 4) * 128 + 128)
                        ktile = kt[j][kc // 4]
                        nc.tensor.matmul(
                            pss[:, 0:512], ktile[0:64, ksl], qt[j][q2][0:64, :],
                            start=True, stop=True,
                        )
                        nc.tensor.matmul(
                            pss[:, 512:1024], ktile[64:128, ksl],
                            qt[j][q2][64:128, :],
                            start=True, stop=True,
                        )
                        pt = sb.tile([128, 1024], BF16, tag="pt", bufs=6)
                        nc.scalar.activation(
                            pt[:, :], pss[:, :],
                            mybir.ActivationFunctionType.Exp,
                        )
                        # PV: col-tiled concurrent pair (h0 -> array cols
                        # 0-63 / psC rows 0-63, h1 -> cols 64-127)
                        if j == 0:
                            vha = vt0[kc][:, 0:64]
                            vhb = vt0[kc][:, 64:128]
                        else:
                            vha = vt123[kc][:, (j - 1) * 128:(j - 1) * 128 + 64]
                            vhb = vt123[kc][:, (j - 1) * 128 + 64:j * 128]
                        nc.tensor.matmul(
                            psC[0:64, :], vha, pt[:, 0:512],
                            start=(kc == 0), stop=(kc == KC - 1),
                            tile_position=(0, 0),
                        )
                        nc.tensor.matmul(
                            psC[64:128, :], vhb, pt[:, 512:1024],
                            start=(kc == 0), stop=(kc == KC - 1),
                            tile_position=(0, 64),
                        )
                        # denominators, same col-tiling trick: rows become
                        # 64 broadcast copies of sum_k P[k, q]
                        nc.tensor.matmul(
                            psD[0:64, :], ones64[:, :], pt[:, 0:512],
                            start=(kc == 0), stop=(kc == KC - 1),
                            tile_position=(0, 0),
                        )
                        nc.tensor.matmul(
                            psD[64:128, :], ones64[:, :], pt[:, 512:1024],
                            start=(kc == 0), stop=(kc == KC - 1),
                            tile_position=(0, 64),
                        )
                        pump(1)
                    # softmax tail on vector engine; scalar stays on Exp
                    # (custom-DVE reciprocal_approx_* doesn't compile in this
                    # walrus; the iterative reciprocal is ~4.3us/tile but DVE
                    # has headroom)
                    bcr = sb.tile([128, 512], F32, tag="bcr", bufs=2)
                    nc.vector.reciprocal(out=bcr[:, :], in_=psD[:, :])
                    with nc.allow_low_precision(reason="fp16 out"):
                        nc.vector.tensor_mul(aot[j][q2][:, :], psC[:, :], bcr[:, :])
                    if j == PAIRS - 1 and q2 < QT - 1:
                        # interleave this 512-query block's output projection
                        # into the NEXT q2's scalar slack, one unit per pump
                        # ("pp" tag only: den/pv slots are live mid-loop)
                        for tc_ in range(4):
                            for nt in range(2):
                                bg.append((f"op{q2}_{tc_}_{nt}",
                                           lambda q2=q2, tc_=tc_, nt=nt:
                                           emit_oproj(q2, tc_, nt)))
            # flush: remaining background + the last q2 block's O-proj,
            # rotated over now-free psum slots for a pipelined tail
            pump(10 ** 6)
            rot = ["pp", "pd", "pv", "pv"]
            for i, (tc_, nt) in enumerate((tc_, nt) for tc_ in range(4)
                                          for nt in range(2)):
                emit_oproj(QT - 1, tc_, nt, tag=rot[i % 4])

    _split_multi_waits(nc)
    return nc


_nc_cache = {}
_last_results = None


def _get_nc():
    if "nc" not in _nc_cache:
        _nc_cache["nc"] = build_bass()
    return _nc_cache["nc"]


def _prep_weights(wq, bq, wk, bk, wv, bv, wo, hh):
    """Per-core (head-half hh) packed weights."""
    c0 = hh * 512  # first out-dim of this head-half
    sc = np.float32(1.0 / np.sqrt(DH))
    wqT = np.ascontiguousarray(wq.T) * sc
    wkT = np.ascontiguousarray(wk.T)
    wvT = np.ascontiguousarray(wv.T)
    woT = np.ascontiguousarray(wo.T)
    # WQP[j, p, d*128 + m] = wqT[d*128 + p, c0 + j*128 + m]
    A = wqT[:, c0:c0 + 512].reshape(DINC, 128, PAIRS, 128)
    WQP = np.ascontiguousarray(A.transpose(2, 1, 0, 3).reshape(PAIRS, 128, 1024)).astype(np.float16)
    A = wkT[:, c0:c0 + 512].reshape(DINC, 128, PAIRS, 128)
    WKP = np.ascontiguousarray(A.transpose(2, 1, 0, 3).reshape(PAIRS, 128, 1024)).astype(np.float16)
    # WVP0[p, d*128 + n] = wvT[d*128 + p, c0 + n]            (n < 128)
    A = wvT[:, c0:c0 + 128].reshape(DINC, 128, 128)
    WVP0 = np.ascontiguousarray(A.transpose(1, 0, 2).reshape(128, 1024)).astype(np.float16)
    # WVP123[p, d*384 + n] = wvT[d*128 + p, c0 + 128 + n]    (n < 384)
    A = wvT[:, c0 + 128:c0 + 512].reshape(DINC, 128, 384)
    WVP123 = np.ascontiguousarray(A.transpose(1, 0, 2).reshape(128, 3072)).astype(np.float16)
    # WOP[nt, p, j*512 + n] = woT[c0 + j*128 + p, nt*512 + n]
    A = woT[c0:c0 + 512, :].reshape(PAIRS, 128, 2, 512)
    WOP = np.ascontiguousarray(A.transpose(2, 1, 0, 3).reshape(2, 128, 2048)).astype(np.float16)
    bqs = (bq[c0:c0 + 512] * sc).reshape(PAIRS, 128)
    bkr = bk[c0:c0 + 512].reshape(PAIRS, 128)
    BQK = np.empty((128, 2 * PAIRS), np.float32)
    for jx in range(PAIRS):
        BQK[:, 2 * jx] = bqs[jx]
        BQK[:, 2 * jx + 1] = bkr[jx]
    BVB = np.ascontiguousarray(np.tile(bv[c0:c0 + 512].reshape(1, 512), (128, 1)))
    return {"WQP": WQP, "WKP": WKP, "WVP0": WVP0, "WVP123": WVP123,
            "WOP": WOP, "BQK": BQK, "BVB": BVB}


def kernel(x_input, wq, bq, wk, bk, wv, bv, wo, bo):
    x_input = np.asarray(x_input, dtype=np.float32)
    wq, bq = np.asarray(wq, np.float32), np.asarray(bq, np.float32)
    wk, bk = np.asarray(wk, np.float32), np.asarray(bk, np.float32)
    wv, bv = np.asarray(wv, np.float32), np.asarray(bv, np.float32)
    wo, bo = np.asarray(wo, np.float32), np.asarray(bo, np.float32)

    halves = [_prep_weights(wq, bq, wk, bk, wv, bv, wo, hh) for hh in range(2)]
    xTs = [np.ascontiguousarray(x_input[b].T).astype(np.float16) for b in range(B)]

    nc = _get_nc()
    in_maps = []
    for c in range(N_CORES):
        b, hh = c // 2, c % 2
        m = dict(halves[hh])
        m["XT"] = xTs[b]
        in_maps.append(m)

    res = run_bass_kernel_spmd(nc, in_maps, list(range(N_CORES)))
    global _last_results
    _last_results = res

    out = np.empty((B, S, D), np.float32)
    for b in range(B):
        out[b] = res.results[2 * b]["Y"]
        out[b] += res.results[2 * b + 1]["Y"]
    out += bo.reshape(1, 1, D)
    return out


# revision 27
# speedup vs baseline: 1.3059x; 1.3059x over previous
"""Distributed MultiHeadAttention kernel for 8 TRN2 NeuronCores.

Problem: B=4, S=2048, D=1024, H=16, DH=64, fp32 reference, full
(non-causal) attention. ~137 GFLOP total.

Sharding (v2): core c owns batch b=c//2 and HEAD-HALF hh=c%2 (8 heads,
full 2048-query x 2048-key attention). Unlike the query-split baseline,
K/V projections are NOT duplicated (each core projects only its own 8
heads' K/V/Q over the full sequence). The output projection produces a
PARTIAL Y[2048,1024] (contraction over the core's 512 ao-dims); the host
sums the two head-half partials per batch and adds bo. One SPMD program;
per-core behavior is entirely encoded in the input tensors (weight
slices per head-half, x^T per batch).

Per-core pipeline (all matmuls fp16/bf16, fp32 PSUM):
- Scores are computed transposed sT[keys, q] per head-pair: h0 contracts
  on partitions 0-63, h1 on 64-127; the two MMs run CONCURRENTLY (PE
  row-group tiling, auto-derived from lhsT base_partition) into one
  [128, 1024] PSUM tile, so ONE scalar Exp per key chunk does the
  softmax numerator (no max subtraction; |s| <~ 35, bf16 P).
- PV uses PE COLUMN-group tiling: h0's V (M=64) lands in array cols
  0-63 and h1's V in cols 64-127 via explicit tile_position=(0,0)/
  (0,64); the two MMs run concurrently, writing aoT for both heads into
  one [128, 512] PSUM tile (h0 rows 0-63, h1 rows 64-127).
- Softmax denominators come from a second col-tiled concurrent MM pair
  with an all-ones [128, 64] stationary: out rows = 64 broadcast copies
  of sum_k P[k, q] per head, accumulated in PSUM across key chunks.
  1/den runs on the VECTOR engine (reciprocal_approx_fast, 18-bit) --
  the scalar engine stays dedicated to Exp (it is the ~276us wall:
  256 x [128,1024] Exp at ~1.077us cadence).
- The attention inner loop needs only ~640ns/kc of tensor time vs
  1077ns/kc of scalar time; projections for later pairs + the V
  projection + per-q2-block output projections are emitted interleaved
  into that slack (pair-0's V projection is split out at N=128 so the
  first exp can issue ~10us into the kernel).
- Output projection runs per 512-query block as soon as pair 3's
  normalize for that block completes, so the tail after the last exp is
  only ~one block of O-proj + DMA.
- walrus in this environment rejects >1 semaphore wait per instruction;
  a post-pass hoists extra waits onto standalone same-engine
  InstEventSemaphore instructions.
"""
import numpy as np
import ml_dtypes
import concourse.bass as bass
import concourse.mybir as mybir
from concourse.tile import TileContext
from concourse.bass_utils import run_bass_kernel_spmd


def _ensure_trace_shim():
    """concourse's axon trace path imports antenv.axon_hooks, which this
    container's antenv lacks. Install a working ctypes-based NTFF hook (or a
    None hook) so BASS_TRACE=1 degrades gracefully instead of crashing."""
    try:
        import antenv.axon_hooks  # noqa: F401
        return
    except ImportError:
        pass
    import sys as _sys
    import types as _types
    hook = None
    try:
        if "/root/.axon_site" not in _sys.path:
            _sys.path.insert(0, "/root/.axon_site")
        from trn_agent_boot.trn_boot import _ntff_profile_via_ctypes
        hook = _ntff_profile_via_ctypes("/opt/axon/libaxon_pjrt.so")
    except Exception:
        hook = None
    mod = _types.ModuleType("antenv.axon_hooks")
    mod.get_axon_ntff_profile_hook = lambda: hook
    mod.set_axon_ntff_profile_hook = lambda h: None
    _sys.modules["antenv.axon_hooks"] = mod
    try:
        import concourse.bass_utils as _bu
        _bu.upload_artifacts = lambda tmpdir: f"local:{tmpdir}"
    except Exception:
        pass


_ensure_trace_shim()


F32 = mybir.dt.float32
F32R = mybir.dt.float32r
BF16 = mybir.dt.bfloat16
FP16 = mybir.dt.float16

B, S, D, H = 4, 2048, 1024, 16
DH = D // H
N_CORES = 8
PAIRS = 4                  # local head pairs per core (8 heads)
DINC = 8                   # 128-wide din chunks
KC = S // 128              # 16 key chunks
QT = S // 512              # 4 query tiles of 512

_ws_counter = 0


def _split_multi_waits(nc):
    """walrus in this env rejects >1 sem wait per instruction; hoist extras
    onto same-engine standalone semaphore-wait instructions."""
    global _ws_counter
    f = nc.m.functions[0]
    for bb in f.blocks:
        insts = bb.instructions  # live list
        i = 0
        while i < len(insts):
            inst = insts[i]
            si = inst.sync_info
            waits = list(si.on_wait) if si is not None and si.on_wait else []
            if len(waits) > 1:
                eng = getattr(inst, "engine", None)
                assert eng is not None and eng in nc.engines, (
                    f"multi-wait on non-engine inst {inst.name} ({type(inst).__name__})"
                )
                for w in waits[:-1]:
                    _ws_counter += 1
                    ev = mybir.InstEventSemaphore(
                        name=f"I-wsplit-{_ws_counter}", ins=[], outs=[]
                    )
                    ev.engine = eng
                    ev.sync_info = mybir.SyncInfo(on_wait=[w], on_update=[])
                    nc.register_instruction(ev, overwrite=True)
                    insts.insert(i, ev)
                    i += 1
                inst.sync_info = mybir.SyncInfo(
                    on_wait=[waits[-1]], on_update=list(si.on_update or [])
                )
            i += 1


def build_bass():
    nc = bass.Bass()
    XT = nc.declare_dram_parameter("XT", [D, S], FP16, isOutput=False)
    WQP = nc.declare_dram_parameter("WQP", [PAIRS, 128, 1024], FP16, isOutput=False)
    WKP = nc.declare_dram_parameter("WKP", [PAIRS, 128, 1024], FP16, isOutput=False)
    WVP0 = nc.declare_dram_parameter("WVP0", [128, 1024], FP16, isOutput=False)
    WVP123 = nc.declare_dram_parameter("WVP123", [128, 3072], FP16, isOutput=False)
    WOP = nc.declare_dram_parameter("WOP", [2, 128, 2048], FP16, isOutput=False)
    BQK = nc.declare_dram_parameter("BQK", [128, 2 * PAIRS], F32, isOutput=False)
    BVB = nc.declare_dram_parameter("BVB", [128, 512], F32, isOutput=False)
    Y = nc.declare_dram_parameter("Y", [S, D], F32, isOutput=True)

    with TileContext(nc) as tc:
        with (
            tc.tile_pool(name="sb", bufs=1) as sb,
            tc.tile_pool(name="ps", bufs=1, space="PSUM") as ps,
        ):
            # ---- constants
            ones64 = sb.tile([128, 64], BF16, tag="ones64")
            nc.vector.memset(ones64[:, :], 1.0)
            junk = sb.tile([128, 512], FP16, tag="junk")
            nc.vector.memset(junk[:, :], 0.0)

            # ---- xT in waves: cols 0-511 first (split across two queues),
            # then 512-1023 and 1024-2047 on gpsimd only, keeping the sync
            # queue free for weight loads.
            xt = []
            for d in range(DINC):
                t = sb.tile([128, S], FP16, tag=f"xt{d}")
                eng = nc.sync if d % 2 == 0 else nc.gpsimd
                eng.dma_start(out=t[:, 0:512], in_=XT[d * 128:(d + 1) * 128, 0:512])
                xt.append(t)
            for d in range(DINC):
                nc.gpsimd.dma_start(out=xt[d][:, 512:1024],
                                    in_=XT[d * 128:(d + 1) * 128, 512:1024])
            for d in range(DINC):
                nc.gpsimd.dma_start(out=xt[d][:, 1024:2048],
                                    in_=XT[d * 128:(d + 1) * 128, 1024:2048])

            # ---- weights (sync queue): startup-critical first
            wq_t = [None] * PAIRS
            wk_t = [None] * PAIRS
            wq_t[0] = sb.tile([128, 1024], FP16, tag="wq", bufs=2, name="wq0")
            wk_t[0] = sb.tile([128, 1024], FP16, tag="wk", bufs=2, name="wk0")
            nc.sync.dma_start(out=wq_t[0][:, :], in_=WQP[0, :, :])
            nc.sync.dma_start(out=wk_t[0][:, :], in_=WKP[0, :, :])
            wv0_t = sb.tile([128, 1024], FP16, tag="wv0", name="wv0_t")
            nc.sync.dma_start(out=wv0_t[:, :], in_=WVP0[:, :])
            bqk = sb.tile([128, 2 * PAIRS], F32, tag="bqk")
            bvb = sb.tile([128, 512], F32, tag="bvb")
            nc.sync.dma_start(out=bqk[:, :], in_=BQK[:, :])
            nc.sync.dma_start(out=bvb[:, :], in_=BVB[:, :])
            wv123_t = sb.tile([128, 3072], FP16, tag="wv123", name="wv123_t")
            nc.sync.dma_start(out=wv123_t[:, :], in_=WVP123[:, :])

            # ---- PE warm-up: ~3.4us of junk matmuls while DMAs land, so the
            # HAM clock gate is at 8/8 when the first real matmul issues (the
            # junk psum is never read; ps_s slots are free until attention)
            for w in range(2):
                jps = ps.tile([128, 1024], F32, tag="ps_s", bufs=2, name=f"jps{w}")
                for i in range(10):
                    nc.tensor.matmul(jps[:, 0:512], junk[:, 0:128], junk[:, :],
                                     start=True, stop=True)

            # V tiles per key chunk: pair0 separate from pairs 1-3 so pair-0
            # attention only depends on the cheap N=128 projection.
            vt0 = [sb.tile([128, 128], BF16, tag=f"vt0_{kc}", name=f"vt0_{kc}") for kc in range(KC)]
            vt123 = [sb.tile([128, 384], BF16, tag=f"vt123_{kc}", name=f"vt123_{kc}") for kc in range(KC)]
            # qt/kt per pair as 4 tiles of [128, 512] (tok granularity)
            qt = [[None] * QT for _ in range(PAIRS)]
            kt = [[None] * QT for _ in range(PAIRS)]
            # aoT per (pair, q2): [128, 512] fp16
            aot = [[sb.tile([128, 512], FP16, tag=f"ao{j}_{q2}", name=f"ao{j}_{q2}")
                    for q2 in range(QT)] for j in range(PAIRS)]

            def emit_vp0(kc, tag="pp"):
                # [128, 512] request keeps every "pp" allocation equal-sized
                vps = ps.tile([128, 512], F32, tag=tag,
                              bufs=2 if tag == "pv" else 1)
                for d in range(DINC):
                    nc.tensor.matmul(
                        vps[:, 0:128], xt[d][:, kc * 128:(kc + 1) * 128],
                        wv0_t[:, d * 128:(d + 1) * 128],
                        start=(d == 0), stop=(d == DINC - 1),
                    )
                with nc.allow_low_precision(reason="bf16 V"):
                    nc.vector.tensor_add(vt0[kc][:, :], vps[:, 0:128], bvb[:, 0:128])

            def emit_vp123(kc):
                vps = ps.tile([128, 512], F32, tag="pp")
                for d in range(DINC):
                    nc.tensor.matmul(
                        vps[:, 0:384], xt[d][:, kc * 128:(kc + 1) * 128],
                        wv123_t[:, d * 384:(d + 1) * 384],
                        start=(d == 0), stop=(d == DINC - 1),
                    )
                with nc.allow_low_precision(reason="bf16 V"):
                    nc.vector.tensor_add(vt123[kc][:, :], vps[:, 0:384], bvb[:, 128:512])

            def emit_qproj(j, t, tag="pp"):
                if wq_t[j] is None:
                    need(f"wdma{j}")
                # qT tile [128 pair-dims, 512 toks]
                qt[j][t] = sb.tile([128, 512], FP16, tag="qt", bufs=2 * QT, name=f"qt{j}_{t}")
                qps = ps.tile([128, 512], F32, tag=tag,
                              bufs=2 if tag == "pv" else 1)
                for d in range(DINC):
                    nc.tensor.matmul(
                        qps[:, :], wq_t[j][:, d * 128:(d + 1) * 128],
                        xt[d][:, t * 512:(t + 1) * 512],
                        start=(d == 0), stop=(d == DINC - 1),
                    )
                with nc.allow_low_precision(reason="f32r rounding"):
                    nc.vector.tensor_scalar_add(
                        qt[j][t][:, :], qps[:, :], bqk[:, 2 * j:2 * j + 1])

            def emit_kproj(j, t, tag="pp"):
                if wk_t[j] is None:
                    need(f"wdma{j}")
                kt[j][t] = sb.tile([128, 512], FP16, tag="kt", bufs=2 * QT, name=f"kt{j}_{t}")
                kps = ps.tile([128, 512], F32, tag=tag,
                              bufs=2 if tag == "pv" else 1)
                for d in range(DINC):
                    nc.tensor.matmul(
                        kps[:, :], wk_t[j][:, d * 128:(d + 1) * 128],
                        xt[d][:, t * 512:(t + 1) * 512],
                        start=(d == 0), stop=(d == DINC - 1),
                    )
                with nc.allow_low_precision(reason="f32r rounding"):
                    nc.vector.tensor_scalar_add(
                        kt[j][t][:, :], kps[:, :], bqk[:, 2 * j + 1:2 * j + 2])

            def emit_wdma(j):
                wq_t[j] = sb.tile([128, 1024], FP16, tag="wq", bufs=2, name=f"wq{j}")
                wk_t[j] = sb.tile([128, 1024], FP16, tag="wk", bufs=2, name=f"wk{j}")
                nc.sync.dma_start(out=wq_t[j][:, :], in_=WQP[j, :, :])
                nc.sync.dma_start(out=wk_t[j][:, :], in_=WKP[j, :, :])

            wo_t = [None, None]

            def emit_wodma():
                for nt in range(2):
                    wo_t[nt] = sb.tile([128, 2048], FP16, tag=f"wo{nt}", name=f"wo{nt}")
                    nc.sync.dma_start(out=wo_t[nt][:, :], in_=WOP[nt, :, :])

            def emit_oproj(q2, tc_, nt, tag="pp"):
                # Y rows [q2*512 + tc_*128 .. +128), cols [nt*512 .. +512)
                yps = ps.tile([128, 512], F32, tag=tag,
                              bufs=2 if tag == "pv" else 1)
                for j in range(PAIRS):
                    nc.tensor.matmul(
                        yps[:, :], aot[j][q2][:, tc_ * 128:(tc_ + 1) * 128],
                        wo_t[nt][:, j * 512:(j + 1) * 512],
                        start=(j == 0), stop=(j == PAIRS - 1),
                    )
                y_sb = sb.tile([128, 512], F32, tag="y", bufs=2)
                nc.vector.tensor_copy(y_sb[:, :], yps[:, :])
                r0 = q2 * 512 + tc_ * 128
                nc.sync.dma_start(out=Y[r0:r0 + 128, nt * 512:(nt + 1) * 512],
                                  in_=y_sb[:, :])

            # ---------- background work queue ----------
            # (key, emit_fn) ordered by when each result is first needed
            # (vt0[kc] at kc, kt0 tile t at kc=4t, qt0 tile t at q2=t,
            # vt123 before pair 1). pump() pushes ~1 item per kc iteration;
            # need() pulls a specific item early (build-time safety: a tile's
            # writers must be emitted before its readers).
            bg = []
            bg.append(("wdma1", lambda: emit_wdma(1)))
            for kc in range(4, KC):
                if kc % 4 == 0:
                    t = kc // 4
                    bg.append((f"k0_{t}", lambda t=t: emit_kproj(0, t)))
                bg.append((f"vp0_{kc}", lambda kc=kc: emit_vp0(kc)))
            for t in range(1, QT):
                bg.append((f"q0_{t}", lambda t=t: emit_qproj(0, t)))
            for kc in range(KC):
                bg.append((f"vp123_{kc}", lambda kc=kc: emit_vp123(kc)))
            bg.append(("wodma", lambda: emit_wodma()))
            # remaining projections are appended per-pair below

            bg_done = set()

            def bg_run(idx):
                key, fn = bg[idx]
                if key not in bg_done:
                    bg_done.add(key)
                    fn()

            bg_i = [0]

            def pump(n=1):
                done = 0
                while bg_i[0] < len(bg) and done < n:
                    if bg[bg_i[0]][0] not in bg_done:
                        bg_run(bg_i[0])
                        done += 1
                    bg_i[0] += 1

            def need(key):
                for idx in range(len(bg)):
                    if bg[idx][0] == key:
                        bg_run(idx)
                        return
                raise KeyError(key)

            # ---------- startup: minimal work before the first exp,
            # pipelined across three free psum tags (no drain stalls)
            emit_vp0(0, tag="pp")
            emit_kproj(0, 0, tag="pv")
            emit_qproj(0, 0, tag="pd")
            emit_vp0(1, tag="pp")
            emit_vp0(2, tag="pv")
            emit_vp0(3, tag="pd")

            # ---------- main attention loops ----------
            for j in range(PAIRS):
                if j + 1 < PAIRS:
                    # queue next pair's projections behind current bg work
                    bg.append((f"k{j+1}_0", lambda j=j: emit_kproj(j + 1, 0)))
                    bg.append((f"q{j+1}_0", lambda j=j: emit_qproj(j + 1, 0)))
                    for t in range(1, QT):
                        bg.append((f"k{j+1}_{t}", lambda j=j, t=t: emit_kproj(j + 1, t)))
                    for t in range(1, QT):
                        bg.append((f"q{j+1}_{t}", lambda j=j, t=t: emit_qproj(j + 1, t)))
                    if j + 2 < PAIRS:
                        bg.append((f"wdma{j+2}", lambda j=j: emit_wdma(j + 2)))
                for q2 in range(QT):
                    # build-time safety pulls (normally already pumped);
                    # per-kc tiles are pulled just-in-time inside the loop
                    if qt[j][q2] is None:
                        need(f"q{j}_{q2}")
                    if j == PAIRS - 1:
                        need("wodma")
                    psC = ps.tile([128, 512], F32, tag="pv", bufs=2)
                    psD = ps.tile([128, 512], F32, tag="pd")
                    qsl = slice(q2 * 512, (q2 + 1) * 512)

                    def emit_pvden(kc, pt):
                        # PV: col-tiled concurrent pair (h0 -> array cols
                        # 0-63 / psC rows 0-63, h1 -> cols 64-127)
                        if j == 0:
                            vha = vt0[kc][:, 0:64]
                            vhb = vt0[kc][:, 64:128]
                        else:
                            vha = vt123[kc][:, (j - 1) * 128:(j - 1) * 128 + 64]
                            vhb = vt123[kc][:, (j - 1) * 128 + 64:j * 128]
                        nc.tensor.matmul(
                            psC[0:64, :], vha, pt[:, 0:512],
                            start=(kc == 0), stop=(kc == KC - 1),
                            tile_position=(0, 0),
                        )
                        nc.tensor.matmul(
                            psC[64:128, :], vhb, pt[:, 512:1024],
                            start=(kc == 0), stop=(kc == KC - 1),
                            tile_position=(0, 64),
                        )
                        # denominators, same col-tiling trick: rows become
                        # 64 broadcast copies of sum_k P[k, q]
                        nc.tensor.matmul(
                            psD[0:64, :], ones64[:, :], pt[:, 0:512],
                            start=(kc == 0), stop=(kc == KC - 1),
                            tile_position=(0, 0),
                        )
                        nc.tensor.matmul(
                            psD[64:128, :], ones64[:, :], pt[:, 512:1024],
                            start=(kc == 0), stop=(kc == KC - 1),
                            tile_position=(0, 64),
                        )

                    # software-pipelined: scores(kc)+exp(kc) get priority
                    # over PV/den(kc-1), so the scalar engine is never
                    # starved by the exp(kc-1)->PV(kc-1) dependency stall
                    pt_prev = None
                    for kc in range(KC):
                        if kt[j][kc // 4] is None:
                            need(f"k{j}_{kc // 4}")
                        if j == 0 and kc >= 4:
                            need(f"vp0_{kc}")
                        elif j == 1:
                            need(f"vp123_{kc}")
                        pss = ps.tile([128, 1024], F32, tag="ps_s", bufs=2)
                        ksl = slice((kc % 4) * 128, (kc % 4) * 128 + 128)
                        ktile = kt[j][kc // 4]
                        nc.tensor.matmul(
                            pss[:, 0:512], ktile[0:64, ksl], qt[j][q2][0:64, :],
                            start=True, stop=True,
                        )
                        nc.tensor.matmul(
                            pss[:, 512:1024], ktile[64:128, ksl],
                            qt[j][q2][64:128, :],
                            start=True, stop=True,
                        )
                        pt = sb.tile([128, 1024], BF16, tag="pt", bufs=6)
                        nc.scalar.activation(
                            pt[:, :], pss[:, :],
                            mybir.ActivationFunctionType.Exp,
                        )
                        if pt_prev is not None:
                            emit_pvden(kc - 1, pt_prev)
                        pt_prev = pt
                        pump(1)
                    emit_pvden(KC - 1, pt_prev)

                    # softmax tail: free psD quickly (copy to SBUF) so the
                    # next q2's den matmuls aren't gated on the ~3.4us DVE
                    # reciprocal; the last unit uses the now-idle scalar
                    # engine (exp(-ln(x))) to keep the tail short
                    bcr = sb.tile([128, 512], F32, tag="bcr", bufs=2)
                    if j == PAIRS - 1 and q2 == QT - 1:
                        lnt = sb.tile([128, 512], F32, tag="lnt")
                        nc.scalar.activation(lnt[:, :], psD[:, :],
                                             mybir.ActivationFunctionType.Ln)
                        nc.scalar.activation(bcr[:, :], lnt[:, :],
                                             mybir.ActivationFunctionType.Exp,
                                             scale=-1.0)
                    else:
                        den_sb = sb.tile([128, 512], F32, tag="den_sb", bufs=2)
                        nc.vector.tensor_copy(den_sb[:, :], psD[:, :])
                        nc.vector.reciprocal(out=bcr[:, :], in_=den_sb[:, :])
                    with nc.allow_low_precision(reason="fp16 out"):
                        nc.vector.tensor_mul(aot[j][q2][:, :], psC[:, :], bcr[:, :])
                    if j == PAIRS - 1 and q2 < QT - 1:
                        # interleave this 512-query block's output projection
                        # into the NEXT q2's scalar slack, one unit per pump
                        # ("pp" tag only: den/pv slots are live mid-loop)
                        for tc_ in range(4):
                            for nt in range(2):
                                bg.append((f"op{q2}_{tc_}_{nt}",
                                           lambda q2=q2, tc_=tc_, nt=nt:
                                           emit_oproj(q2, tc_, nt)))
            # flush: remaining background + the last q2 block's O-proj.
            # Split each psum tile's accumulation: pairs 0-2 don't depend on
            # the final normalize, so those matmuls run during the softmax
            # tail (also keeping the PE's HAM clock-gate warm); only the
            # pair-3 matmul + drain wait for aot[3][3].
            pump(10 ** 6)
            rot = ["pp", "pv", "pv", "pd"]
            units = [(tc_, nt) for tc_ in range(4) for nt in range(2)]
            for half in range(2):
                ypss = []
                for i, (tc_, nt) in enumerate(units[half * 4:half * 4 + 4]):
                    yps = ps.tile([128, 512], F32, tag=rot[i],
                                  bufs=2 if rot[i] == "pv" else 1,
                                  name=f"yfl{half}_{i}")
                    for j in range(PAIRS - 1):
                        nc.tensor.matmul(
                            yps[:, :], aot[j][QT - 1][:, tc_ * 128:(tc_ + 1) * 128],
                            wo_t[nt][:, j * 512:(j + 1) * 512],
                            start=(j == 0), stop=False,
                        )
                    ypss.append(yps)
                for i, (tc_, nt) in enumerate(units[half * 4:half * 4 + 4]):
                    jl = PAIRS - 1
                    nc.tensor.matmul(
                        ypss[i][:, :],
                        aot[jl][QT - 1][:, tc_ * 128:(tc_ + 1) * 128],
                        wo_t[nt][:, jl * 512:(jl + 1) * 512],
                        start=False, stop=True,
                    )
                    y_sb = sb.tile([128, 512], F32, tag="y", bufs=2)
                    nc.vector.tensor_copy(y_sb[:, :], ypss[i][:, :])
                    r0 = (QT - 1) * 512 + tc_ * 128
                    nc.sync.dma_start(
                        out=Y[r0:r0 + 128, nt * 512:(nt + 1) * 512],
                        in_=y_sb[:, :])

    _split_multi_waits(nc)
    return nc


_nc_cache = {}
_last_results = None


def _get_nc():
    if "nc" not in _nc_cache:
        _nc_cache["nc"] = build_bass()
    return _nc_cache["nc"]


def _prep_weights(wq, bq, wk, bk, wv, bv, wo, hh):
    """Per-core (head-half hh) packed weights."""
    c0 = hh * 512  # first out-dim of this head-half
    sc = np.float32(1.0 / np.sqrt(DH))
    wqT = np.ascontiguousarray(wq.T) * sc
    wkT = np.ascontiguousarray(wk.T)
    wvT = np.ascontiguousarray(wv.T)
    woT = np.ascontiguousarray(wo.T)
    # WQP[j, p, d*128 + m] = wqT[d*128 + p, c0 + j*128 + m]
    A = wqT[:, c0:c0 + 512].reshape(DINC, 128, PAIRS, 128)
    WQP = np.ascontiguousarray(A.transpose(2, 1, 0, 3).reshape(PAIRS, 128, 1024)).astype(np.float16)
    A = wkT[:, c0:c0 + 512].reshape(DINC, 128, PAIRS, 128)
    WKP = np.ascontiguousarray(A.transpose(2, 1, 0, 3).reshape(PAIRS, 128, 1024)).astype(np.float16)
    # WVP0[p, d*128 + n] = wvT[d*128 + p, c0 + n]            (n < 128)
    A = wvT[:, c0:c0 + 128].reshape(DINC, 128, 128)
    WVP0 = np.ascontiguousarray(A.transpose(1, 0, 2).reshape(128, 1024)).astype(np.float16)
    # WVP123[p, d*384 + n] = wvT[d*128 + p, c0 + 128 + n]    (n < 384)
    A = wvT[:, c0 + 128:c0 + 512].reshape(DINC, 128, 384)
    WVP123 = np.ascontiguousarray(A.transpose(1, 0, 2).reshape(128, 3072)).astype(np.float16)
    # WOP[nt, p, j*512 + n] = woT[c0 + j*128 + p, nt*512 + n]
    A = woT[c0:c0 + 512, :].reshape(PAIRS, 128, 2, 512)
    WOP = np.ascontiguousarray(A.transpose(2, 1, 0, 3).reshape(2, 128, 2048)).astype(np.float16)
    bqs = (bq[c0:c0 + 512] * sc).reshape(PAIRS, 128)
    bkr = bk[c0:c0 + 512].reshape(PAIRS, 128)
    BQK = np.empty((128, 2 * PAIRS), np.float32)
    for jx in range(PAIRS):
        BQK[:, 2 * jx] = bqs[jx]
        BQK[:, 2 * jx + 1] = bkr[jx]
    BVB = np.ascontiguousarray(np.tile(bv[c0:c0 + 512].reshape(1, 512), (128, 1)))
    return {"WQP": WQP, "WKP": WKP, "WVP0": WVP0, "WVP123": WVP123,
            "WOP": WOP, "BQK": BQK, "BVB": BVB}


def kernel(x_input, wq, bq, wk, bk, wv, bv, wo, bo):
    x_input = np.asarray(x_input, dtype=np.float32)
    wq, bq = np.asarray(wq, np.float32), np.asarray(bq, np.float32)
    wk, bk = np.asarray(wk, np.float32), np.asarray(bk, np.float32)
    wv, bv = np.asarray(wv, np.float32), np.asarray(bv, np.float32)
    wo, bo = np.asarray(wo, np.float32), np.asarray(bo, np.float32)

    halves = [_prep_weights(wq, bq, wk, bk, wv, bv, wo, hh) for hh in range(2)]
    xTs = [np.ascontiguousarray(x_input[b].T).astype(np.float16) for b in range(B)]

    nc = _get_nc()
    in_maps = []
    for c in range(N_CORES):
        b, hh = c // 2, c % 2
        m = dict(halves[hh])
        m["XT"] = xTs[b]
        in_maps.append(m)

    res = run_bass_kernel_spmd(nc, in_maps, list(range(N_CORES)))
    global _last_results
    _last_results = res

    out = np.empty((B, S, D), np.float32)
    for b in range(B):
        out[b] = res.results[2 * b]["Y"]
        out[b] += res.results[2 * b + 1]["Y"]
    out += bo.reshape(1, 1, D)
    return out


# revision 30
# speedup vs baseline: 1.3603x; 1.0417x over previous
"""Distributed MultiHeadAttention kernel for 8 TRN2 NeuronCores.

Problem: B=4, S=2048, D=1024, H=16, DH=64, fp32 reference, full
(non-causal) attention. ~137 GFLOP total.

Sharding (v2): core c owns batch b=c//2 and HEAD-HALF hh=c%2 (8 heads,
full 2048-query x 2048-key attention). Unlike the query-split baseline,
K/V projections are NOT duplicated (each core projects only its own 8
heads' K/V/Q over the full sequence). The output projection produces a
PARTIAL Y[2048,1024] (contraction over the core's 512 ao-dims); the host
sums the two head-half partials per batch and adds bo. One SPMD program;
per-core behavior is entirely encoded in the input tensors (weight
slices per head-half, x^T per batch).

Per-core pipeline (all matmuls fp16/bf16, fp32 PSUM):
- Scores are computed transposed sT[keys, q] per head-pair: h0 contracts
  on partitions 0-63, h1 on 64-127; the two MMs run CONCURRENTLY (PE
  row-group tiling, auto-derived from lhsT base_partition) into one
  [128, 1024] PSUM tile, so ONE scalar Exp per key chunk does the
  softmax numerator (no max subtraction; |s| <~ 35, bf16 P).
- PV uses PE COLUMN-group tiling: h0's V (M=64) lands in array cols
  0-63 and h1's V in cols 64-127 via explicit tile_position=(0,0)/
  (0,64); the two MMs run concurrently, writing aoT for both heads into
  one [128, 512] PSUM tile (h0 rows 0-63, h1 rows 64-127).
- Softmax denominators come from a second col-tiled concurrent MM pair
  with an all-ones [128, 64] stationary: out rows = 64 broadcast copies
  of sum_k P[k, q] per head, accumulated in PSUM across key chunks.
  1/den runs on the VECTOR engine (reciprocal_approx_fast, 18-bit) --
  the scalar engine stays dedicated to Exp (it is the ~276us wall:
  256 x [128,1024] Exp at ~1.077us cadence).
- The attention inner loop needs only ~640ns/kc of tensor time vs
  1077ns/kc of scalar time; projections for later pairs + the V
  projection + per-q2-block output projections are emitted interleaved
  into that slack (pair-0's V projection is split out at N=128 so the
  first exp can issue ~10us into the kernel).
- Output projection runs per 512-query block as soon as pair 3's
  normalize for that block completes, so the tail after the last exp is
  only ~one block of O-proj + DMA.
- walrus in this environment rejects >1 semaphore wait per instruction;
  a post-pass hoists extra waits onto standalone same-engine
  InstEventSemaphore instructions.
"""
import numpy as np
import ml_dtypes
import concourse.bass as bass
import concourse.mybir as mybir
from concourse.tile import TileContext
from concourse.bass_utils import run_bass_kernel_spmd


def _ensure_trace_shim():
    """concourse's axon trace path imports antenv.axon_hooks, which this
    container's antenv lacks. Install a working ctypes-based NTFF hook (or a
    None hook) so BASS_TRACE=1 degrades gracefully instead of crashing."""
    try:
        import antenv.axon_hooks  # noqa: F401
        return
    except ImportError:
        pass
    import sys as _sys
    import types as _types
    hook = None
    try:
        if "/root/.axon_site" not in _sys.path:
            _sys.path.insert(0, "/root/.axon_site")
        from trn_agent_boot.trn_boot import _ntff_profile_via_ctypes
        hook = _ntff_profile_via_ctypes("/opt/axon/libaxon_pjrt.so")
    except Exception:
        hook = None
    mod = _types.ModuleType("antenv.axon_hooks")
    mod.get_axon_ntff_profile_hook = lambda: hook
    mod.set_axon_ntff_profile_hook = lambda h: None
    _sys.modules["antenv.axon_hooks"] = mod
    try:
        import concourse.bass_utils as _bu
        _bu.upload_artifacts = lambda tmpdir: f"local:{tmpdir}"
    except Exception:
        pass


_ensure_trace_shim()


F32 = mybir.dt.float32
F32R = mybir.dt.float32r
BF16 = mybir.dt.bfloat16
FP16 = mybir.dt.float16

B, S, D, H = 4, 2048, 1024, 16
DH = D // H
N_CORES = 8
PAIRS = 4                  # local head pairs per core (8 heads)
DINC = 8                   # 128-wide din chunks
KC = S // 128              # 16 key chunks
QT = S // 512              # 4 query tiles of 512

_ws_counter = 0


def _split_multi_waits(nc):
    """walrus in this env rejects >1 sem wait per instruction; hoist extras
    onto same-engine standalone semaphore-wait instructions."""
    global _ws_counter
    f = nc.m.functions[0]
    for bb in f.blocks:
        insts = bb.instructions  # live list
        i = 0
        while i < len(insts):
            inst = insts[i]
            si = inst.sync_info
            waits = list(si.on_wait) if si is not None and si.on_wait else []
            if len(waits) > 1:
                eng = getattr(inst, "engine", None)
                assert eng is not None and eng in nc.engines, (
                    f"multi-wait on non-engine inst {inst.name} ({type(inst).__name__})"
                )
                for w in waits[:-1]:
                    _ws_counter += 1
                    ev = mybir.InstEventSemaphore(
                        name=f"I-wsplit-{_ws_counter}", ins=[], outs=[]
                    )
                    ev.engine = eng
                    ev.sync_info = mybir.SyncInfo(on_wait=[w], on_update=[])
                    nc.register_instruction(ev, overwrite=True)
                    insts.insert(i, ev)
                    i += 1
                inst.sync_info = mybir.SyncInfo(
                    on_wait=[waits[-1]], on_update=list(si.on_update or [])
                )
            i += 1


def build_bass():
    nc = bass.Bass()
    XT = nc.declare_dram_parameter("XT", [D, S], FP16, isOutput=False)
    WQP = nc.declare_dram_parameter("WQP", [PAIRS, 128, 1024], FP16, isOutput=False)
    WKP = nc.declare_dram_parameter("WKP", [PAIRS, 128, 1024], FP16, isOutput=False)
    WVP0 = nc.declare_dram_parameter("WVP0", [128, 1024], FP16, isOutput=False)
    WVP123 = nc.declare_dram_parameter("WVP123", [128, 3072], FP16, isOutput=False)
    WOP = nc.declare_dram_parameter("WOP", [2, 128, 2048], FP16, isOutput=False)
    BQK = nc.declare_dram_parameter("BQK", [128, 2 * PAIRS], F32, isOutput=False)
    BVB = nc.declare_dram_parameter("BVB", [128, 512], F32, isOutput=False)
    Y = nc.declare_dram_parameter("Y", [S, D], F32, isOutput=True)

    with TileContext(nc) as tc:
        with (
            tc.tile_pool(name="sb", bufs=1) as sb,
            tc.tile_pool(name="ps", bufs=1, space="PSUM") as ps,
        ):
            # ---- constants
            ones64 = sb.tile([128, 64], BF16, tag="ones64")
            nc.vector.memset(ones64[:, :], 1.0)
            junk = sb.tile([128, 512], FP16, tag="junk")
            nc.vector.memset(junk[:, :], 0.0)

            # ---- xT in waves: cols 0-511 first (split across two queues),
            # then 512-1023 and 1024-2047 on gpsimd only, keeping the sync
            # queue free for weight loads.
            xt = []
            for d in range(DINC):
                t = sb.tile([128, S], FP16, tag=f"xt{d}")
                eng = nc.sync if d % 2 == 0 else nc.gpsimd
                eng.dma_start(out=t[:, 0:512], in_=XT[d * 128:(d + 1) * 128, 0:512])
                xt.append(t)
            for d in range(DINC):
                nc.gpsimd.dma_start(out=xt[d][:, 512:1024],
                                    in_=XT[d * 128:(d + 1) * 128, 512:1024])
            for d in range(DINC):
                nc.gpsimd.dma_start(out=xt[d][:, 1024:2048],
                                    in_=XT[d * 128:(d + 1) * 128, 1024:2048])

            # ---- weights (sync queue): startup-critical first
            wq_t = [None] * PAIRS
            wk_t = [None] * PAIRS
            wq_t[0] = sb.tile([128, 1024], FP16, tag="wq", bufs=2, name="wq0")
            wk_t[0] = sb.tile([128, 1024], FP16, tag="wk", bufs=2, name="wk0")
            nc.sync.dma_start(out=wq_t[0][:, :], in_=WQP[0, :, :])
            nc.sync.dma_start(out=wk_t[0][:, :], in_=WKP[0, :, :])
            wv0_t = sb.tile([128, 1024], FP16, tag="wv0", name="wv0_t")
            nc.sync.dma_start(out=wv0_t[:, :], in_=WVP0[:, :])
            bqk = sb.tile([128, 2 * PAIRS], F32, tag="bqk")
            bvb = sb.tile([128, 512], F32, tag="bvb")
            nc.sync.dma_start(out=bqk[:, :], in_=BQK[:, :])
            nc.sync.dma_start(out=bvb[:, :], in_=BVB[:, :])
            wv123_t = sb.tile([128, 3072], FP16, tag="wv123", name="wv123_t")
            nc.sync.dma_start(out=wv123_t[:, :], in_=WVP123[:, :])

            # ---- PE warm-up: ~3.4us of junk matmuls while DMAs land, so the
            # HAM clock gate is at 8/8 when the first real matmul issues (the
            # junk psum is never read; ps_s slots are free until attention)
            for w in range(2):
                jps = ps.tile([128, 1024], F32, tag="ps_s", bufs=2, name=f"jps{w}")
                for i in range(10):
                    nc.tensor.matmul(jps[:, 0:512], junk[:, 0:128], junk[:, :],
                                     start=True, stop=True)

            # V tiles per key chunk: pair0 separate from pairs 1-3 so pair-0
            # attention only depends on the cheap N=128 projection.
            vt0 = [sb.tile([128, 128], BF16, tag=f"vt0_{kc}", name=f"vt0_{kc}") for kc in range(KC)]
            vt123 = [sb.tile([128, 384], BF16, tag=f"vt123_{kc}", name=f"vt123_{kc}") for kc in range(KC)]
            # qt/kt per pair as 4 tiles of [128, 512] (tok granularity)
            qt = [[None] * QT for _ in range(PAIRS)]
            kt = [[None] * QT for _ in range(PAIRS)]
            # aoT per (pair, q2): [128, 512] fp16
            aot = [[sb.tile([128, 512], FP16, tag=f"ao{j}_{q2}", name=f"ao{j}_{q2}")
                    for q2 in range(QT)] for j in range(PAIRS)]

            def emit_vp0(kc, tag="pp"):
                # generator: one piece per pump; [128, 512] psum request keeps
                # every "pp" allocation equal-sized
                vps = ps.tile([128, 512], F32, tag=tag,
                              bufs=2 if tag == "pv" else 1)
                for d in range(DINC):
                    nc.tensor.matmul(
                        vps[:, 0:128], xt[d][:, kc * 128:(kc + 1) * 128],
                        wv0_t[:, d * 128:(d + 1) * 128],
                        start=(d == 0), stop=(d == DINC - 1),
                    )
                with nc.allow_low_precision(reason="bf16 V"):
                    nc.vector.tensor_add(vt0[kc][:, :], vps[:, 0:128], bvb[:, 0:128])
                yield

            def emit_vp123(kc):
                vps = ps.tile([128, 512], F32, tag="pp")
                for d in range(DINC):
                    nc.tensor.matmul(
                        vps[:, 0:384], xt[d][:, kc * 128:(kc + 1) * 128],
                        wv123_t[:, d * 384:(d + 1) * 384],
                        start=(d == 0), stop=(d == DINC - 1),
                    )
                    if d == 3:
                        yield
                with nc.allow_low_precision(reason="bf16 V"):
                    nc.vector.tensor_add(vt123[kc][:, :], vps[:, 0:384], bvb[:, 128:512])
                yield

            def emit_qproj(j, t, tag="pp"):
                if wq_t[j] is None:
                    need(f"wdma{j}")
                # qT tile [128 pair-dims, 512 toks]
                qt[j][t] = sb.tile([128, 512], FP16, tag="qt", bufs=2 * QT, name=f"qt{j}_{t}")
                qps = ps.tile([128, 512], F32, tag=tag,
                              bufs=2 if tag == "pv" else 1)
                for d in range(DINC):
                    nc.tensor.matmul(
                        qps[:, :], wq_t[j][:, d * 128:(d + 1) * 128],
                        xt[d][:, t * 512:(t + 1) * 512],
                        start=(d == 0), stop=(d == DINC - 1),
                    )
                    if d % 2 == 1 and d < DINC - 1:
                        yield
                with nc.allow_low_precision(reason="f32r rounding"):
                    nc.vector.tensor_scalar_add(
                        qt[j][t][:, :], qps[:, :], bqk[:, 2 * j:2 * j + 1])
                yield

            def emit_kproj(j, t, tag="pp"):
                if wk_t[j] is None:
                    need(f"wdma{j}")
                kt[j][t] = sb.tile([128, 512], FP16, tag="kt", bufs=2 * QT, name=f"kt{j}_{t}")
                kps = ps.tile([128, 512], F32, tag=tag,
                              bufs=2 if tag == "pv" else 1)
                for d in range(DINC):
                    nc.tensor.matmul(
                        kps[:, :], wk_t[j][:, d * 128:(d + 1) * 128],
                        xt[d][:, t * 512:(t + 1) * 512],
                        start=(d == 0), stop=(d == DINC - 1),
                    )
                    if d % 2 == 1 and d < DINC - 1:
                        yield
                with nc.allow_low_precision(reason="f32r rounding"):
                    nc.vector.tensor_scalar_add(
                        kt[j][t][:, :], kps[:, :], bqk[:, 2 * j + 1:2 * j + 2])
                yield

            def emit_wdma(j):
                wq_t[j] = sb.tile([128, 1024], FP16, tag="wq", bufs=2, name=f"wq{j}")
                wk_t[j] = sb.tile([128, 1024], FP16, tag="wk", bufs=2, name=f"wk{j}")
                nc.sync.dma_start(out=wq_t[j][:, :], in_=WQP[j, :, :])
                nc.sync.dma_start(out=wk_t[j][:, :], in_=WKP[j, :, :])
                yield

            wo_t = [None, None]

            def emit_wodma():
                for nt in range(2):
                    wo_t[nt] = sb.tile([128, 2048], FP16, tag=f"wo{nt}", name=f"wo{nt}")
                    nc.sync.dma_start(out=wo_t[nt][:, :], in_=WOP[nt, :, :])
                yield

            def emit_oproj(q2, tc_, nt, tag="pp"):
                # Y rows [q2*512 + tc_*128 .. +128), cols [nt*512 .. +512)
                yps = ps.tile([128, 512], F32, tag=tag,
                              bufs=2 if tag == "pv" else 1)
                for j in range(PAIRS):
                    nc.tensor.matmul(
                        yps[:, :], aot[j][q2][:, tc_ * 128:(tc_ + 1) * 128],
                        wo_t[nt][:, j * 512:(j + 1) * 512],
                        start=(j == 0), stop=(j == PAIRS - 1),
                    )
                    if j == 1:
                        yield
                y_sb = sb.tile([128, 512], F32, tag="y", bufs=2)
                nc.vector.tensor_copy(y_sb[:, :], yps[:, :])
                r0 = q2 * 512 + tc_ * 128
                nc.sync.dma_start(out=Y[r0:r0 + 128, nt * 512:(nt + 1) * 512],
                                  in_=y_sb[:, :])
                yield

            # ---------- background work queue ----------
            # (key, emit_fn) ordered by when each result is first needed
            # (vt0[kc] at kc, kt0 tile t at kc=4t, qt0 tile t at q2=t,
            # vt123 before pair 1). pump() pushes ~1 item per kc iteration;
            # need() pulls a specific item early (build-time safety: a tile's
            # writers must be emitted before its readers).
            bg = []
            bg.append(("wdma1", lambda: emit_wdma(1)))
            for kc in range(4, KC):
                if kc % 4 == 0:
                    t = kc // 4
                    bg.append((f"k0_{t}", lambda t=t: emit_kproj(0, t)))
                bg.append((f"vp0_{kc}", lambda kc=kc: emit_vp0(kc)))
            for t in range(1, QT):
                bg.append((f"q0_{t}", lambda t=t: emit_qproj(0, t)))
            for kc in range(KC):
                bg.append((f"vp123_{kc}", lambda kc=kc: emit_vp123(kc)))
            bg.append(("wodma", lambda: emit_wodma()))
            # remaining projections are appended per-pair below

            # each bg entry is (key, generator_factory); pump() advances one
            # PIECE (a few matmuls) per call so background work never inserts
            # a >0.5us priority block ahead of the next scores pair; need()
            # runs an item to completion (build-time dependency safety)
            bg_done = set()
            bg_gens = {}
            bg_i = [0]

            def _advance(key, gf, full=False):
                g = bg_gens.get(key)
                if g is None:
                    g = bg_gens[key] = gf()
                while True:
                    try:
                        next(g)
                    except StopIteration:
                        bg_done.add(key)
                        return True
                    if not full:
                        return False

            def pump(n=1):
                done = 0
                while bg_i[0] < len(bg) and done < n:
                    key, gf = bg[bg_i[0]]
                    if key in bg_done:
                        bg_i[0] += 1
                        continue
                    finished = _advance(key, gf)
                    done += 1
                    if finished:
                        bg_i[0] += 1

            def need(key):
                if key in bg_done:
                    return
                for idx in range(len(bg)):
                    if bg[idx][0] == key:
                        _advance(key, bg[idx][1], full=True)
                        return
                raise KeyError(key)

            def run_now(gen):
                for _ in gen:
                    pass

            # ---------- startup: minimal work before the first exp,
            # pipelined across three free psum tags (no drain stalls)
            run_now(emit_vp0(0, tag="pp"))
            run_now(emit_kproj(0, 0, tag="pv"))
            run_now(emit_qproj(0, 0, tag="pd"))
            run_now(emit_vp0(1, tag="pp"))
            run_now(emit_vp0(2, tag="pv"))
            run_now(emit_vp0(3, tag="pd"))
            bg_done.update({"k0_0", "q0_0", "vp0_0", "vp0_1", "vp0_2", "vp0_3"})

            # ---------- main attention loops ----------
            for j in range(PAIRS):
                if j + 1 < PAIRS:
                    # queue next pair's projections behind current bg work
                    bg.append((f"k{j+1}_0", lambda j=j: emit_kproj(j + 1, 0)))
                    bg.append((f"q{j+1}_0", lambda j=j: emit_qproj(j + 1, 0)))
                    for t in range(1, QT):
                        bg.append((f"k{j+1}_{t}", lambda j=j, t=t: emit_kproj(j + 1, t)))
                    for t in range(1, QT):
                        bg.append((f"q{j+1}_{t}", lambda j=j, t=t: emit_qproj(j + 1, t)))
                    if j + 2 < PAIRS:
                        bg.append((f"wdma{j+2}", lambda j=j: emit_wdma(j + 2)))
                for q2 in range(QT):
                    # build-time safety pulls (normally already pumped).
                    # NOTE: completion must be checked via bg_done -- a
                    # partially-advanced generator leaves the tile allocated
                    # but not yet fully written
                    if f"q{j}_{q2}" not in bg_done:
                        need(f"q{j}_{q2}")
                    if j == PAIRS - 1:
                        need("wodma")
                    psC = ps.tile([128, 512], F32, tag="pv", bufs=2)
                    psD = ps.tile([128, 512], F32, tag="pd")
                    qsl = slice(q2 * 512, (q2 + 1) * 512)

                    def emit_pvden(kc, pt):
                        # PV: col-tiled concurrent pair (h0 -> array cols
                        # 0-63 / psC rows 0-63, h1 -> cols 64-127)
                        if j == 0:
                            vha = vt0[kc][:, 0:64]
                            vhb = vt0[kc][:, 64:128]
                        else:
                            vha = vt123[kc][:, (j - 1) * 128:(j - 1) * 128 + 64]
                            vhb = vt123[kc][:, (j - 1) * 128 + 64:j * 128]
                        nc.tensor.matmul(
                            psC[0:64, :], vha, pt[:, 0:512],
                            start=(kc == 0), stop=(kc == KC - 1),
                            tile_position=(0, 0),
                        )
                        nc.tensor.matmul(
                            psC[64:128, :], vhb, pt[:, 512:1024],
                            start=(kc == 0), stop=(kc == KC - 1),
                            tile_position=(0, 64),
                        )
                        # denominators, same col-tiling trick: rows become
                        # 64 broadcast copies of sum_k P[k, q]
                        nc.tensor.matmul(
                            psD[0:64, :], ones64[:, :], pt[:, 0:512],
                            start=(kc == 0), stop=(kc == KC - 1),
                            tile_position=(0, 0),
                        )
                        nc.tensor.matmul(
                            psD[64:128, :], ones64[:, :], pt[:, 512:1024],
                            start=(kc == 0), stop=(kc == KC - 1),
                            tile_position=(0, 64),
                        )

                    # software-pipelined: scores(kc)+exp(kc) get priority
                    # over PV/den(kc-1), so the scalar engine is never
                    # starved by the exp(kc-1)->PV(kc-1) dependency stall
                    pt_prev = None
                    for kc in range(KC):
                        if f"k{j}_{kc // 4}" not in bg_done:
                            need(f"k{j}_{kc // 4}")
                        if j == 0 and kc >= 4:
                            need(f"vp0_{kc}")
                        elif j == 1:
                            need(f"vp123_{kc}")
                        pss = ps.tile([128, 1024], F32, tag="ps_s", bufs=2)
                        ksl = slice((kc % 4) * 128, (kc % 4) * 128 + 128)
                        ktile = kt[j][kc // 4]
                        nc.tensor.matmul(
                            pss[:, 0:512], ktile[0:64, ksl], qt[j][q2][0:64, :],
                            start=True, stop=True,
                        )
                        nc.tensor.matmul(
                            pss[:, 512:1024], ktile[64:128, ksl],
                            qt[j][q2][64:128, :],
                            start=True, stop=True,
                        )
                        pt = sb.tile([128, 1024], BF16, tag="pt", bufs=6)
                        nc.scalar.activation(
                            pt[:, :], pss[:, :],
                            mybir.ActivationFunctionType.Exp,
                        )
                        if pt_prev is not None:
                            emit_pvden(kc - 1, pt_prev)
                        pt_prev = pt
                        pump(1)
                    emit_pvden(KC - 1, pt_prev)

                    # softmax tail: 1/x = exp(-ln(x)) on the SCALAR engine
                    # (~1.2us; scalar has slack since the tensor engine is
                    # now the wall, and Ln frees psD fast for the next q2's
                    # den matmuls -- the DVE iterative reciprocal is 3.4us
                    # and was congesting the vector engine)
                    bcr = sb.tile([128, 512], F32, tag="bcr", bufs=2)
                    lnt = sb.tile([128, 512], F32, tag="lnt", bufs=2)
                    nc.scalar.activation(lnt[:, :], psD[:, :],
                                         mybir.ActivationFunctionType.Ln)
                    nc.scalar.activation(bcr[:, :], lnt[:, :],
                                         mybir.ActivationFunctionType.Exp,
                                         scale=-1.0)
                    with nc.allow_low_precision(reason="fp16 out"):
                        nc.vector.tensor_mul(aot[j][q2][:, :], psC[:, :], bcr[:, :])
                    if j == PAIRS - 1 and q2 < QT - 1:
                        # interleave this 512-query block's output projection
                        # into the NEXT q2's scalar slack, one unit per pump
                        # ("pp" tag only: den/pv slots are live mid-loop)
                        for tc_ in range(4):
                            for nt in range(2):
                                bg.append((f"op{q2}_{tc_}_{nt}",
                                           lambda q2=q2, tc_=tc_, nt=nt:
                                           emit_oproj(q2, tc_, nt)))
            # flush: remaining background + the last q2 block's O-proj.
            # Split each psum tile's accumulation: pairs 0-2 don't depend on
            # the final normalize, so those matmuls run during the softmax
            # tail (also keeping the PE's HAM clock-gate warm); only the
            # pair-3 matmul + drain wait for aot[3][3].
            pump(10 ** 6)
            rot = ["pp", "pv", "pv", "pd"]
            # (flush O-proj below is emitted inline, not via bg)
            units = [(tc_, nt) for tc_ in range(4) for nt in range(2)]
            for half in range(2):
                ypss = []
                for i, (tc_, nt) in enumerate(units[half * 4:half * 4 + 4]):
                    yps = ps.tile([128, 512], F32, tag=rot[i],
                                  bufs=2 if rot[i] == "pv" else 1,
                                  name=f"yfl{half}_{i}")
                    for j in range(PAIRS - 1):
                        nc.tensor.matmul(
                            yps[:, :], aot[j][QT - 1][:, tc_ * 128:(tc_ + 1) * 128],
                            wo_t[nt][:, j * 512:(j + 1) * 512],
                            start=(j == 0), stop=False,
                        )
                    ypss.append(yps)
                for i, (tc_, nt) in enumerate(units[half * 4:half * 4 + 4]):
                    jl = PAIRS - 1
                    nc.tensor.matmul(
                        ypss[i][:, :],
                        aot[jl][QT - 1][:, tc_ * 128:(tc_ + 1) * 128],
                        wo_t[nt][:, jl * 512:(jl + 1) * 512],
                        start=False, stop=True,
                    )
                    y_sb = sb.tile([128, 512], F32, tag="y", bufs=2)
                    nc.vector.tensor_copy(y_sb[:, :], ypss[i][:, :])
                    r0 = (QT - 1) * 512 + tc_ * 128
                    nc.sync.dma_start(
                        out=Y[r0:r0 + 128, nt * 512:(nt + 1) * 512],
                        in_=y_sb[:, :])

    _split_multi_waits(nc)
    return nc


_nc_cache = {}
_last_results = None


def _get_nc():
    if "nc" not in _nc_cache:
        _nc_cache["nc"] = build_bass()
    return _nc_cache["nc"]


def _prep_weights(wq, bq, wk, bk, wv, bv, wo, hh):
    """Per-core (head-half hh) packed weights."""
    c0 = hh * 512  # first out-dim of this head-half
    sc = np.float32(1.0 / np.sqrt(DH))
    wqT = np.ascontiguousarray(wq.T) * sc
    wkT = np.ascontiguousarray(wk.T)
    wvT = np.ascontiguousarray(wv.T)
    woT = np.ascontiguousarray(wo.T)
    # WQP[j, p, d*128 + m] = wqT[d*128 + p, c0 + j*128 + m]
    A = wqT[:, c0:c0 + 512].reshape(DINC, 128, PAIRS, 128)
    WQP = np.ascontiguousarray(A.transpose(2, 1, 0, 3).reshape(PAIRS, 128, 1024)).astype(np.float16)
    A = wkT[:, c0:c0 + 512].reshape(DINC, 128, PAIRS, 128)
    WKP = np.ascontiguousarray(A.transpose(2, 1, 0, 3).reshape(PAIRS, 128, 1024)).astype(np.float16)
    # WVP0[p, d*128 + n] = wvT[d*128 + p, c0 + n]            (n < 128)
    A = wvT[:, c0:c0 + 128].reshape(DINC, 128, 128)
    WVP0 = np.ascontiguousarray(A.transpose(1, 0, 2).reshape(128, 1024)).astype(np.float16)
    # WVP123[p, d*384 + n] = wvT[d*128 + p, c0 + 128 + n]    (n < 384)
    A = wvT[:, c0 + 128:c0 + 512].reshape(DINC, 128, 384)
    WVP123 = np.ascontiguousarray(A.transpose(1, 0, 2).reshape(128, 3072)).astype(np.float16)
    # WOP[nt, p, j*512 + n] = woT[c0 + j*128 + p, nt*512 + n]
    A = woT[c0:c0 + 512, :].reshape(PAIRS, 128, 2, 512)
    WOP = np.ascontiguousarray(A.transpose(2, 1, 0, 3).reshape(2, 128, 2048)).astype(np.float16)
    bqs = (bq[c0:c0 + 512] * sc).reshape(PAIRS, 128)
    bkr = bk[c0:c0 + 512].reshape(PAIRS, 128)
    BQK = np.empty((128, 2 * PAIRS), np.float32)
    for jx in range(PAIRS):
        BQK[:, 2 * jx] = bqs[jx]
        BQK[:, 2 * jx + 1] = bkr[jx]
    BVB = np.ascontiguousarray(np.tile(bv[c0:c0 + 512].reshape(1, 512), (128, 1)))
    return {"WQP": WQP, "WKP": WKP, "WVP0": WVP0, "WVP123": WVP123,
            "WOP": WOP, "BQK": BQK, "BVB": BVB}


def kernel(x_input, wq, bq, wk, bk, wv, bv, wo, bo):
    x_input = np.asarray(x_input, dtype=np.float32)
    wq, bq = np.asarray(wq, np.float32), np.asarray(bq, np.float32)
    wk, bk = np.asarray(wk, np.float32), np.asarray(bk, np.float32)
    wv, bv = np.asarray(wv, np.float32), np.asarray(bv, np.float32)
    wo, bo = np.asarray(wo, np.float32), np.asarray(bo, np.float32)

    halves = [_prep_weights(wq, bq, wk, bk, wv, bv, wo, hh) for hh in range(2)]
    xTs = [np.ascontiguousarray(x_input[b].T).astype(np.float16) for b in range(B)]

    nc = _get_nc()
    in_maps = []
    for c in range(N_CORES):
        b, hh = c // 2, c % 2
        m = dict(halves[hh])
        m["XT"] = xTs[b]
        in_maps.append(m)

    res = run_bass_kernel_spmd(nc, in_maps, list(range(N_CORES)))
    global _last_results
    _last_results = res

    out = np.empty((B, S, D), np.float32)
    for b in range(B):
        out[b] = res.results[2 * b]["Y"]
        out[b] += res.results[2 * b + 1]["Y"]
    out += bo.reshape(1, 1, D)
    return out
